# revision 11
# baseline (speedup 1.0000x reference)
"""Trainium2 Bass kernel for the MoE-Adapter module.

Math (per token):
  y = x @ W_base + b_base                       (dense base, stop-grad)
    + (x @ A_s) @ B_s                           (shared rank-16 LoRA)
    + sum_e w_e * (x @ A_r[e]) @ B_r[e]         (6 routed rank-16 LoRA experts)
  w = softmax(x @ W_router + b_router) masked to its top-2 entries

Strategy: data-parallel over the 16384 tokens across 8 NeuronCores (2048
tokens/core); all weights replicated.  Compute in fp16 (full PE rate,
fp32 PSUM accumulation).  The router is computed in fp16-pair precision
(x = xh + xl, W_router = Wrh + Wrl, logits = xh@Wrh + xh@Wrl + xl@Wrh)
so top-2 selection matches the fp32 reference.

Per-core layout: the host stages x as an fp16 pair in transposed,
token-tile-major layout (xhT/xlT: [tile, din%128, ktile*128+tok]) and
W_base as fp16, so every device-side load is one contiguous DMA (the
HWDGE direct2d descriptor only has room for 2 sync-wait commands, which
rules out multi-dependency staging chains on-chip).  The rank-space
projections of all 7 experts plus both router column groups are packed
into one 128-wide rhs so the whole adapter+router down-projection is a
single accumulation chain.  Gate weights are applied in rank space, the
scaled rank vectors are transposed on the PE and folded into the base
matmul's PSUM accumulation together with the bias (K=1 ones matmul).
"""

import os
import sys

import numpy as np

for _p in ("/opt/trn_rl_repo",):
    if os.path.isdir(_p) and _p not in sys.path:
        sys.path.insert(0, _p)

import concourse.bass as bass
import concourse.mybir as mybir
import concourse.tile as tile
from concourse import bacc
from concourse import bass_utils
from concourse.masks import make_identity

B, S, D, E, R = 4, 4096, 2048, 6, 16
NCORES = 8
NTOK = B * S               # 16384 tokens total
P = 128
KT = D // P                # 16 k-tiles over the contraction dim
NCHUNK = 512               # PSUM bank width (fp32)
NCH = D // NCHUNK          # 4 output column chunks
NEG = -60000.0             # exp() flushes this to 0; fits in fp16

F32 = mybir.dt.float32
F16 = mybir.dt.float16


def build_kernel(T: int) -> bacc.Bacc:
    """Build the per-core kernel for T tokens (T % 128 == 0)."""
    TT = T // P
    nc = bacc.Bacc("TRN2", target_bir_lowering=False, debug=False)

    xhT_d = nc.dram_tensor("xhT", [T // P, P, D], F16, kind="ExternalInput").ap()
    xlT_d = nc.dram_tensor("xlT", [T // P, P, D], F16, kind="ExternalInput").ap()
    Wb_d = nc.dram_tensor("W16", [D, D], F16, kind="ExternalInput").ap()
    bb_d = nc.dram_tensor("b_base", [1, D], F32, kind="ExternalInput").ap()
    As_d = nc.dram_tensor("A_s", [D, R], F32, kind="ExternalInput").ap()
    Bs_d = nc.dram_tensor("B_s", [R, D], F32, kind="ExternalInput").ap()
    Ar_d = nc.dram_tensor("A_r", [E, D, R], F32, kind="ExternalInput").ap()
    Br_d = nc.dram_tensor("B_r", [E, R, D], F32, kind="ExternalInput").ap()
    Wr_d = nc.dram_tensor("W_router", [D, E], F32, kind="ExternalInput").ap()
    br_d = nc.dram_tensor("b_router", [1, E], F32, kind="ExternalInput").ap()
    y_d = nc.dram_tensor("y", [T, D], F32, kind="ExternalOutput").ap()

    with tile.TileContext(nc) as tc:
        with (
            tc.tile_pool(name="const", bufs=1) as const,
            tc.tile_pool(name="wpool", bufs=1) as wpool,
            tc.tile_pool(name="small", bufs=1) as small,
            tc.tile_pool(name="xT", bufs=3) as xTp,
            tc.tile_pool(name="gate", bufs=2) as gate,
            tc.tile_pool(name="yout", bufs=2) as yout,
            tc.tile_pool(name="psY", bufs=3, space="PSUM") as psY,
            tc.tile_pool(name="psH", bufs=2, space="PSUM") as psH,
            tc.tile_pool(name="psT", bufs=2, space="PSUM") as psT,
        ):
            # ---- constants ----
            ones = const.tile([1, P], F16)
            nc.vector.memset(ones[:], 1.0)
            ident = const.tile([P, P], F16)
            make_identity(nc, ident[:])

            # brow: K=1 bias row for the adapter/router chain
            # [0]*112 | b_router (6) | NEG pads (2) | 0 (8)
            brow = const.tile([1, P], F16)
            nc.vector.memset(brow[:], 0.0)
            brs = small.tile([1, E], F32, tag="brs")
            nc.sync.dma_start(brs[:], br_d[:])
            nc.vector.tensor_copy(brow[:, 112:118], brs[:])
            nc.vector.memset(brow[:, 118:120], NEG)

            # b_base as fp16 K=1 matmul rhs
            bb16 = const.tile([1, D], F16)
            bbs = small.tile([1, D], F32, tag="bbs")
            nc.sync.dma_start(bbs[:], bb_d[:])
            nc.vector.tensor_copy(bb16[:], bbs[:])

            # ---- adapter down-proj + router rhs: [P, KT, 128] fp16 ----
            # cols: A_r[e]*6 (96) | A_s(16) | Wrh(6) | 0(2) | Wrl(6) | 0(2)
            AR = const.tile([P, KT, P], F16)
            nc.vector.memset(AR[:, :, 118:120], 0.0)
            nc.vector.memset(AR[:, :, 126:128], 0.0)
            for e in range(E):
                art = small.tile([P, KT, R], F32, tag=f"art{e}")
                nc.sync.dma_start(art[:], Ar_d[e].rearrange("(k p) r -> p k r", p=P))
                nc.vector.tensor_copy(AR[:, :, 16 * e:16 * (e + 1)], art[:])
            ast = small.tile([P, KT, R], F32, tag="ast")
            nc.sync.dma_start(ast[:], As_d.rearrange("(k p) r -> p k r", p=P))
            nc.vector.tensor_copy(AR[:, :, 96:112], ast[:])
            wrt = small.tile([P, KT, E], F32, tag="wrt")
            nc.sync.dma_start(wrt[:], Wr_d.rearrange("(k p) e -> p k e", p=P))
            nc.vector.tensor_copy(AR[:, :, 112:118], wrt[:])
            # Wrl = fp32(Wr) - fp16(Wr), rounded to fp16
            nc.vector.tensor_sub(AR[:, :, 120:126], wrt[:], AR[:, :, 112:118])

            # ---- up-proj weights: [112, D] fp16 ----
            Bc = const.tile([112, D], F16)
            bst = small.tile([112, D], F32, tag="bst")
            nc.sync.dma_start(bst[0:96, :], Br_d.rearrange("e r d -> (e r) d"))
            nc.sync.dma_start(bst[96:112, :], Bs_d[:])
            # two 32-aligned casts so each waits on only one DMA-queue sem
            nc.scalar.activation(Bc[0:96, :], bst[0:96, :],
                                 mybir.ActivationFunctionType.Copy)
            nc.scalar.activation(Bc[96:112, :], bst[96:112, :],
                                 mybir.ActivationFunctionType.Copy)

            # ---- base weight fp16 k-tiles (host pre-cast to fp16) ----
            Wk = []
            for kt in range(KT):
                wk = wpool.tile([P, D], F16, tag=f"w{kt}")
                nc.sync.dma_start(wk[:], Wb_d[kt * P:(kt + 1) * P, :])
                Wk.append(wk)

            # ---- main loop over 128-token tiles ----
            for t in range(TT):
                xhT = xTp.tile([P, D], F16, tag="xhT")
                nc.sync.dma_start(xhT[:], xhT_d[t])
                xlT = xTp.tile([P, D], F16, tag="xlT")
                nc.sync.dma_start(xlT[:], xlT_d[t])

                # stage 1: rank-space projections + router logits
                psh = psH.tile([P, P], F32)
                for kt in range(KT):
                    nc.tensor.matmul(psh[:], xhT[:, kt * P:(kt + 1) * P],
                                     AR[:, kt, :], start=(kt == 0), stop=False)
                for kt in range(KT):
                    nc.tensor.matmul(psh[:, 112:118], xlT[:, kt * P:(kt + 1) * P],
                                     AR[:, kt, 112:118], start=False, stop=False,
                                     skip_group_check=True)
                nc.tensor.matmul(psh[:], ones[:], brow[:], start=False, stop=True)

                # stage 2: top-2 gating  w = softmax(L) * (L >= secondmax(L))
                Lsb = gate.tile([P, 8], F32, tag="Lsb")
                nc.vector.tensor_copy(Lsb[:], psh[:, 112:120])
                nc.vector.tensor_add(Lsb[:, 0:6], Lsb[:, 0:6], psh[:, 120:126])
                M8 = gate.tile([P, 8], F32, tag="M8")
                nc.vector.max(out=M8[:], in_=Lsb[:])
                nm1 = gate.tile([P, 1], F32, tag="nm1")
                nc.vector.tensor_scalar_mul(nm1[:], M8[:, 0:1], -1.0)
                es = gate.tile([P, 8], F32, tag="es")
                ssum = gate.tile([P, 1], F32, tag="ssum")
                nc.scalar.activation(es[:], Lsb[:], mybir.ActivationFunctionType.Exp,
                                     bias=nm1[:], accum_out=ssum[:])
                rcp = gate.tile([P, 1], F32, tag="rcp")
                nc.vector.reciprocal(rcp[:], ssum[:])
                msk = gate.tile([P, 8], F32, tag="msk")
                nc.vector.tensor_scalar(msk[:], Lsb[:], M8[:, 1:2], scalar2=None,
                                        op0=mybir.AluOpType.is_ge)
                wgt = gate.tile([P, 8], F32, tag="wgt")
                nc.vector.scalar_tensor_tensor(wgt[:], es[:], rcp[:], msk[:],
                                               op0=mybir.AluOpType.mult,
                                               op1=mybir.AluOpType.mult)

                # stage 3: scale rank vectors by gate weights (shared block is 1x)
                sfull = gate.tile([P, 96], F32, tag="sfull")
                for e in range(E):
                    nc.vector.tensor_copy(sfull[:, 16 * e:16 * (e + 1)],
                                          wgt[:, e:e + 1].to_broadcast([P, 16]))
                Hs16 = gate.tile([P, 112], F16, tag="Hs16")
                nc.vector.tensor_mul(Hs16[:, 0:96], psh[:, 0:96], sfull[:])
                nc.vector.tensor_copy(Hs16[:, 96:112], psh[:, 96:112])

                # stage 4: transpose scaled rank vectors -> [112, 128] fp16
                pst = psT.tile([112, P], F16)
                nc.tensor.transpose(pst[:], Hs16[:], ident[:])
                HsT = gate.tile([112, P], F16, tag="HsT")
                nc.vector.tensor_copy(HsT[:], pst[:])

                # stage 5: base matmul + adapter up-proj + bias, fused in PSUM
                ysb = yout.tile([P, D], F32, tag="ysb")
                for n in range(NCH):
                    psy = psY.tile([P, NCHUNK], F32)
                    lo = n * NCHUNK
                    for kt in range(KT):
                        nc.tensor.matmul(psy[:], xhT[:, kt * P:(kt + 1) * P],
                                         Wk[kt][:, lo:lo + NCHUNK],
                                         start=(kt == 0), stop=False)
                    nc.tensor.matmul(psy[:], HsT[:], Bc[:, lo:lo + NCHUNK],
                                     start=False, stop=False)
                    nc.tensor.matmul(psy[:], ones[:], bb16[:, lo:lo + NCHUNK],
                                     start=False, stop=True)
                    nc.scalar.activation(ysb[:, lo:lo + NCHUNK], psy[:],
                                         mybir.ActivationFunctionType.Copy)
                nc.sync.dma_start(y_d[t * P:(t + 1) * P, :], ysb[:])

    nc.compile()
    return nc


_cache: dict[int, bacc.Bacc] = {}


def _get_nc(T: int) -> bacc.Bacc:
    if T not in _cache:
        _cache[T] = build_kernel(T)
    return _cache[T]


def _pack_xT(xs: np.ndarray) -> np.ndarray:
    """[T, D] -> [T//P, P, D] with packed[t, p, kt*P + tok] = xs[t*P+tok, kt*P+p]."""
    TT = xs.shape[0] // P
    v = xs.reshape(TT, P, KT, P).transpose(0, 3, 2, 1)
    return np.ascontiguousarray(v).reshape(TT, P, D)


def kernel(**inputs: np.ndarray) -> np.ndarray:
    x = np.ascontiguousarray(np.asarray(inputs["x"], dtype=np.float32).reshape(NTOK, D))
    T = NTOK // NCORES
    xh = x.astype(np.float16)
    xl = (x - xh.astype(np.float32)).astype(np.float16)
    shards = [(xh[i * T:(i + 1) * T], xl[i * T:(i + 1) * T]) for i in range(NCORES)]
    common = {
        "W16": np.ascontiguousarray(np.asarray(inputs["W_base"]).astype(np.float16)),
        "b_base": np.ascontiguousarray(inputs["b_base"], dtype=np.float32).reshape(1, D),
        "A_s": np.ascontiguousarray(inputs["A_s"], dtype=np.float32),
        "B_s": np.ascontiguousarray(inputs["B_s"], dtype=np.float32),
        "A_r": np.ascontiguousarray(inputs["A_r"], dtype=np.float32),
        "B_r": np.ascontiguousarray(inputs["B_r"], dtype=np.float32),
        "W_router": np.ascontiguousarray(inputs["W_router"], dtype=np.float32),
        "b_router": np.ascontiguousarray(inputs["b_router"], dtype=np.float32).reshape(1, E),
    }
    in_maps = [dict(common, xhT=_pack_xT(sh), xlT=_pack_xT(sl))
               for sh, sl in shards]
    nc = _get_nc(T)
    res = bass_utils.run_bass_kernel_spmd(nc, in_maps, core_ids=list(range(NCORES)))
    out = np.concatenate([res.results[i]["y"] for i in range(NCORES)], axis=0)
    return out.reshape(B, S, D)


if __name__ == "__main__":
    rng = np.random.default_rng(0)
    demo = {
        "x": rng.standard_normal((B, S, D), dtype=np.float32),
        "W_base": 0.02 * rng.standard_normal((D, D), dtype=np.float32),
        "b_base": 0.02 * rng.standard_normal((D,), dtype=np.float32),
        "A_s": 0.02 * rng.standard_normal((D, R), dtype=np.float32),
        "B_s": 0.02 * rng.standard_normal((R, D), dtype=np.float32),
        "A_r": 0.02 * rng.standard_normal((E, D, R), dtype=np.float32),
        "B_r": 0.02 * rng.standard_normal((E, R, D), dtype=np.float32),
        "W_router": 0.02 * rng.standard_normal((D, E), dtype=np.float32),
        "b_router": 0.02 * rng.standard_normal((E,), dtype=np.float32),
    }
    y = kernel(**demo)
    print("kernel ran, output", y.shape, y.dtype)
